# revision 1
# baseline (speedup 1.0000x reference)
"""Trainium2 Bass kernel for nn_CAAN_84112639525649 (CAAN dense transformer), v2.

Shapes: B=16, N=512, D_IN=256, D=64. Data-parallel: 2 batches/core on 8 cores.

Design notes (vs v1 baseline):
- The pairwise rank-distance MLP collapses to a 51-entry table
  f[d] = sigmoid(relu(Eemb[d] @ Wr1 + br1) @ Wr2); f depends only on weights,
  so f, g'[k] = f[clip(|k-511|,0,50)] - f[50], and the three Toeplitz tiles
  T_s[a,b] = g'[511 + 128 s + a - b] (s in {-1,0,1}) are host-folded, like the
  score bilinear M1 = [Wq;bq] [Wk;bk]^T / 8.
- rel[i,j] = f50 + g'[r_i - r_j + 511] is computed as two one-hot matmul
  chains: T1_v = sum_w T_{w-v} @ RT_w (10 MMs), rel_c = sum_v RT_v^T @ T1_v
  (16 MMs), with f50 folded into the T1 PSUM->SBUF evacuation.
- Attention is computed fully transposed (P^T = rel^T * S^T with rel
  symmetric), so A^T = exp(P^T) feeds attn@V directly -- no A transposes.
  The softmax row sums come free as row 64 of the AV matmul via an appended
  ones column on V.
- Single activation-function set (natural_log_exp): sigmoid via exp + DVE
  reciprocal, LN rstd via exp(-0.5 ln(var+eps)) batched over all 4 chunks.
- x is shipped host-transposed in fp16; everything on the PE runs fp16
  (1 cyc/row) with f32 PSUM accumulation.
"""

import sys
import os

for _p in ("/opt/trn_rl_repo",):
    if os.path.isdir(_p) and _p not in sys.path:
        sys.path.insert(0, _p)

import numpy as np
from contextlib import ExitStack

import concourse.bass as bass
import concourse.tile as tile
from concourse import bacc, mybir
from concourse.bass import ts

N_CORES = 8
B = 16
PB = B // N_CORES
N = 512
D_IN = 256
D = 64
MAXD = 50
LN_EPS = 1e-5

f32 = mybir.dt.float32
f16 = mybir.dt.float16
AF = mybir.ActivationFunctionType
OP = mybir.AluOpType

# ---- weight packs (fp16 [128, *]): name -> (col, rows, cols) ----
# A: needed by the attention front (T1/scores); B: FFN-side, arrives later.
WPACK_A = {
    "Tm1":   (0,    128, 128),
    "T0":    (128,  128, 128),
    "Tp1":   (256,  128, 128),
    "Wp0":   (384,  128, 64),
    "Wp1":   (448,  128, 64),
    "M1T":   (512,  65,  65),
    "Wv_a":  (577,  65,  64),
}
WPACK32_A = {
    "bp":    (642,  64,  1),
    "f50":   (644,  128, 1),
    "m8":    (646,  128, 1),
}
WFA = 648
WPACK_B = {
    "Wf1":   (0,    64,  128),
    "Wf2":   (128,  128, 64),
    "Ws1p":  (192,  64,  32),
    "Ws2":   (224,  32,  1),
    "ident": (225,  128, 128),
}
WPACK32_B = {
    "bf1":   (354,  128, 1),
    "bs1p":  (356,  32,  1),
    "nbs2":  (358,  128, 1),
    "eps":   (360,  128, 1),
}
WFB = 362

# rowpack (fp16 [1, RF]): ones | r0 | r1 | bf2row | bf1row
ROW_ONES = 0
ROW_R0 = 512
ROW_R1 = 1024
ROW_BF2 = 1536
ROW_BF1 = 1600
RF = 1728

BANDS = {0: (0, 1), 1: (0, 1, 2), 2: (1, 2, 3), 3: (2, 3)}  # w terms per v


def build_nc():
    nc = bacc.Bacc(
        "TRN2",
        target_bir_lowering=False,
        debug=False,
        enable_asserts=False,
        num_devices=N_CORES,
    )

    xT_d = nc.dram_tensor("xT", (PB, 2, 128, N), f16, kind="ExternalInput")
    wpkA_d = nc.dram_tensor("wpackA", (128, WFA), f16, kind="ExternalInput")
    wpkB_d = nc.dram_tensor("wpackB", (128, WFB), f16, kind="ExternalInput")
    rpk_d = nc.dram_tensor("rowpack", (1, RF), f16, kind="ExternalInput")
    out_d = nc.dram_tensor("out", (PB, N), f32, kind="ExternalOutput")

    out_ap = out_d.ap()

    with ExitStack() as ctx:
        tc = ctx.enter_context(tile.TileContext(nc))
        cp = ctx.enter_context(tc.tile_pool(name="consts", bufs=1))
        wk = ctx.enter_context(tc.tile_pool(name="work", bufs=1))
        ps = ctx.enter_context(tc.tile_pool(name="ps", bufs=4, space="PSUM"))

        PS_BUFS = {"big": 4, "io": 2, "sm": 2}

        def psum(shape, tag, dtype=f32):
            return ps.tile(shape, dtype, tag=tag, name="ps_" + tag,
                           bufs=PS_BUFS[tag])

        # ---------------- input DMAs ----------------
        # rowpack (tiny, gates the one-hot chain) first; x for batch 0 next;
        # weights and batch-1 x on the Act queue.
        rpk = cp.tile([1, RF], f16, tag="rpk")
        nc.sync.dma_start(rpk[:, :], rpk_d.ap())
        wpkA = cp.tile([128, WFA], f16, tag="wpkA")
        nc.scalar.dma_start(wpkA[:, :], wpkA_d.ap())
        xall = []
        for b in range(PB):
            xt = wk.tile([128, 2, N], f16, tag=f"xT{b}", name="xT")
            eng = nc.sync if b == 0 else nc.scalar
            eng.dma_start(
                xt[:, :, :],
                bass.AP(tensor=xT_d, offset=b * 2 * 128 * N,
                        ap=[[N, 128], [128 * N, 2], [1, N]]),
            )
            xall.append(xt)
        wpkB = cp.tile([128, WFB], f16, tag="wpkB")
        nc.scalar.dma_start(wpkB[:, :], wpkB_d.ap())

        def wsl(name):
            if name in WPACK_A:
                col, rows, cols = WPACK_A[name]
                return wpkA[0:rows, col: col + cols]
            col, rows, cols = WPACK_B[name]
            return wpkB[0:rows, col: col + cols]

        def wsl32(name):
            if name in WPACK32_A:
                col, rows, cols = WPACK32_A[name]
                return wpkA[0:rows, col: col + 2 * cols].bitcast(f32)
            col, rows, cols = WPACK32_B[name]
            return wpkB[0:rows, col: col + 2 * cols].bitcast(f32)

        Wp = [wsl("Wp0"), wsl("Wp1")]
        M1T = wsl("M1T")
        Wv_a = wsl("Wv_a")
        Wf1 = wsl("Wf1")
        Wf2 = wsl("Wf2")
        Ws1p = wsl("Ws1p")
        Ws2 = wsl("Ws2")
        Ttile = {-1: wsl("Tm1"), 0: wsl("T0"), 1: wsl("Tp1")}
        ident = wsl("ident")
        i32 = mybir.dt.int32
        iota4 = cp.tile([128, 4], f32, tag="iota4")
        for w in range(4):
            it = cp.tile([128, 1], i32, tag=f"it{w}", name="it")
            nc.gpsimd.iota(it[:, :], pattern=[[0, 1]], base=128 * w,
                           channel_multiplier=1)
            nc.gpsimd.tensor_copy(iota4[:, w: w + 1], it[:, :])
        bp_c = wsl32("bp")
        bf1_c = wsl32("bf1")
        bs1p_c = wsl32("bs1p")
        nbs2_c = wsl32("nbs2")
        f50_c = wsl32("f50")
        m8_c = wsl32("m8")
        eps_c = wsl32("eps")
        ones_r = rpk[0:1, ROW_ONES: ROW_ONES + N]
        bf2_r = rpk[0:1, ROW_BF2: ROW_BF2 + D]
        bf1_r = rpk[0:1, ROW_BF1: ROW_BF1 + 128]

        S = [dict() for _ in range(PB)]

        # ---------------- stages ----------------
        def st_onehot(b):
            rrow = rpk[0:1, (ROW_R0 if b == 0 else ROW_R1):][:, :N]
            r_bc = wk.tile([128, N], f16, tag=f"rbc{b}", name="rbc")
            nc.gpsimd.partition_broadcast(r_bc[:, :], rrow)
            S[b]["RT"] = []
            for w in range(4):
                rt = wk.tile([128, N], f16, tag=f"RT{b}_{w}", name="rt")
                eng = nc.gpsimd if w != 1 else nc.vector
                eng.tensor_scalar(rt[:, :], r_bc[:, :],
                                  iota4[:, w: w + 1], None,
                                  op0=OP.is_equal)
                S[b]["RT"].append(rt)

        def st_xp(b):
            xp = wk.tile([65, N], f16, tag=f"xp{b}", name="xp")
            xpp = psum([D, N], tag="io")
            for h in range(2):
                nc.tensor.matmul(xpp[:, :], Wp[h], xall[b][:, h, :],
                                 start=(h == 0), stop=(h == 1))
            nc.scalar.activation(xp[0:D, :], xpp[:, :], AF.Identity,
                                 bias=bp_c, scale=1.0)
            nc.gpsimd.tensor_copy(xp[D: D + 1, :], ones_r)
            S[b]["xp"] = xp

        def st_uv(b):
            xp = S[b]["xp"]
            up = psum([65, N], tag="io")
            nc.tensor.matmul(up[:, :], M1T, xp[:, :], start=True, stop=True)
            uT = wk.tile([65, N], f16, tag=f"uT{b}", name="uT")
            nc.scalar.activation(uT[:, :], up[:, :], AF.Copy)
            S[b]["uT"] = uT
            S[b]["v"] = []
            for c in range(4):
                vp = psum([128, D], tag="sm")
                nc.tensor.matmul(vp[:, :], xp[0:65, ts(c, 128)], Wv_a,
                                 start=True, stop=True)
                v = wk.tile([128, 65], f16, tag=f"v{b}_{c}", name="v")
                if c % 2 == 0:
                    nc.scalar.activation(v[:, 0:D], vp[:, :], AF.Copy)
                else:
                    nc.vector.tensor_copy(v[:, 0:D], vp[:, :])
                nc.gpsimd.memset(v[:, D: D + 1], 1.0)
                S[b]["v"].append(v)

        def st_t1(b):
            S[b]["T1"] = []
            for v in range(4):
                t1p = psum([128, N], tag="big")
                ws = BANDS[v]
                for wi, w in enumerate(ws):
                    nc.tensor.matmul(t1p[:, :], Ttile[w - v], S[b]["RT"][w],
                                     start=(wi == 0), stop=(wi == len(ws) - 1))
                t1 = wk.tile([128, N], f16, tag=f"T1{b}_{v}", name="t1")
                if v < 2:
                    nc.vector.tensor_copy(t1[:, :], t1p[:, :])
                else:
                    nc.scalar.activation(t1[:, :], t1p[:, :], AF.Copy)
                S[b]["T1"].append(t1)

        def st_attn(b, c):
            xp = S[b]["xp"]
            sp = psum([128, N], tag="big")
            nc.tensor.matmul(sp[:, :], xp[0:65, ts(c, 128)], S[b]["uT"],
                             start=True, stop=True)
            relp = psum([128, N], tag="big")
            for v in range(4):
                nc.tensor.matmul(relp[:, :], S[b]["RT"][v][:, ts(c, 128)],
                                 S[b]["T1"][v], start=(v == 0), stop=(v == 3))
            ssb = wk.tile([128, N], f16, tag=f"S{b}_{c}", name="ssb")
            if c != 3:
                nc.scalar.activation(ssb[:, :], sp[:, :], AF.Copy)
            else:
                nc.vector.tensor_copy(ssb[:, :], sp[:, :])
            pm = wk.tile([128, N], f16, tag=f"A{b}_{c}", name="pm")
            nc.vector.scalar_tensor_tensor(pm[:, :], relp[:, :],
                                           f50_c[:, 0:1], ssb[:, :],
                                           op0=OP.add, op1=OP.mult)
            nc.scalar.activation(pm[:, :], pm[:, :], AF.Exp,
                                 bias=m8_c, scale=1.0)
            S[b].setdefault("A", []).append(pm)

        def st_av(b):
            aop = psum([65, N], tag="io")
            for c in range(4):
                nc.tensor.matmul(aop[:, :], S[b]["v"][c], S[b]["A"][c],
                                 start=(c == 0), stop=(c == 3))
            # sigma (row D of aop) is NOT divided out: LayerNorm is invariant
            # to a positive per-token scale, so sigma rides through h = FFN(ao)
            # as long as the bf1/bf2 bias rank-1 matmuls are scaled by sigma
            # too (ones_row -> sigma_row below).
            aoT = wk.tile([D, N], f16, tag=f"aoT{b}", name="aoT")
            nc.vector.tensor_copy(aoT[:, :], aop[0:D, :])
            sg = wk.tile([1, N], f16, tag=f"sg{b}", name="sg")
            nc.scalar.activation(sg[:, :], aop[D: D + 1, :], AF.Copy)
            S[b]["aoT"] = aoT
            S[b]["sg"] = sg

        def st_ffn_h(b):
            sg = S[b]["sg"]
            h1p = psum([128, N], tag="big")
            nc.tensor.matmul(h1p[:, :], Wf1, S[b]["aoT"], start=True,
                             stop=False)
            nc.tensor.matmul(h1p[:, :], bf1_r, sg[0:1, :], start=False,
                             stop=True)
            h1 = wk.tile([128, N], f16, tag=f"h1{b}", name="h1")
            nc.scalar.activation(h1[:, :], h1p[:, :], AF.Relu)
            hp = psum([128, 4 * D], tag="sm")
            for c in range(4):
                nc.tensor.matmul(hp[:, ts(c, D)], h1[:, ts(c, 128)], Wf2,
                                 start=True, stop=False)
                nc.tensor.matmul(hp[:, ts(c, D)], sg[0:1, ts(c, 128)], bf2_r,
                                 start=False, stop=True)
            S[b]["hp"] = hp

        def st_ffn_stats(b):
            hp = S[b]["hp"]
            mv = wk.tile([128, 8], f32, tag=f"mv{b}", name="mv")
            for c in range(4):
                stats = wk.tile([128, 6], f32, tag=f"st{b}_{c}", name="sts")
                nc.vector.bn_stats(stats[:, :], hp[:, ts(c, D)])
                nc.vector.bn_aggr(mv[:, 2 * c: 2 * c + 2], stats[:, :])
            lnv = wk.tile([128, 4], f32, tag=f"lnv{b}", name="lnv")
            var_ap = bass.AP(tensor=mv.tensor, offset=mv.offset + 1,
                             ap=[[mv.ap[0][0], 128], [2, 4]])
            nc.scalar.activation(lnv[:, :], var_ap, AF.Ln,
                                 bias=eps_c, scale=1.0)
            rstd = wk.tile([128, 4], f32, tag=f"rstd{b}", name="rstd")
            nc.scalar.activation(rstd[:, :], lnv[:, :], AF.Exp, scale=-0.5)
            S[b]["mv"] = mv
            S[b]["rstd"] = rstd

        def st_ffn_zo(b):
            hp, mv, rstd = S[b]["hp"], S[b]["mv"], S[b]["rstd"]
            ztp = psum([D, N], tag="io", dtype=f16)
            for c in range(4):
                z = wk.tile([128, D], f16, tag=f"z{b}_{c}", name="z")
                nc.vector.tensor_scalar(z[:, :], hp[:, ts(c, D)],
                                        mv[:, 2 * c: 2 * c + 1],
                                        rstd[:, c: c + 1],
                                        op0=OP.subtract, op1=OP.mult)
                nc.tensor.transpose(ztp[:, ts(c, 128)], z[:, :], ident)
            zT = wk.tile([D, N], f16, tag=f"zT{b}", name="zT")
            if b == 0:
                nc.scalar.activation(zT[:, :], ztp[:, :], AF.Copy)
            else:
                nc.vector.tensor_copy(zT[:, :], ztp[:, :])
            s1p = psum([32, N], tag="io")
            nc.tensor.matmul(s1p[:, :], Ws1p, zT[:, :], start=True, stop=True)
            s1 = wk.tile([32, N], f16, tag=f"s1{b}", name="s1")
            nc.scalar.activation(s1[:, :], s1p[:, :], AF.Relu,
                                 bias=bs1p_c, scale=1.0)
            op_ = psum([128, 4], tag="sm")
            for c in range(4):
                nc.tensor.matmul(op_[:, c: c + 1], s1[:, ts(c, 128)], Ws2,
                                 start=True, stop=True)
            oe = wk.tile([128, 4], f32, tag=f"oe{b}", name="oe")
            nc.scalar.activation(oe[:, :], op_[:, :], AF.Exp,
                                 bias=nbs2_c, scale=-1.0)
            nc.vector.tensor_scalar(oe[:, :], oe[:, :], 1.0, None, op0=OP.add)
            osb = wk.tile([128, 4], f32, tag=f"osb{b}", name="osb")
            nc.vector.reciprocal(osb[:, :], oe[:, :])
            nc.sync.dma_start(
                bass.AP(tensor=out_d, offset=b * N,
                        ap=[[1, 128], [128, 4]]),
                osb[:, :])

        # ---------------- emission order ----------------
        # PE warmup: junk transposes from t~0.3us keep the PE p-state ramp
        # running during the input DMAs so real matmuls start at full clock.
        warm = cp.tile([128, 128], f16, tag="warm")
        nc.vector.memset(warm[:, :], 1.0)
        wps = psum([128, 128], tag="sm", dtype=f16)
        for _ in range(36):
            nc.tensor.transpose(wps[:, :], warm[:, :], warm[:, :])

        st_onehot(0)
        st_t1(0)
        st_xp(0)
        st_onehot(1)
        st_uv(0)
        st_attn(0, 0)
        st_attn(0, 1)
        st_xp(1)
        st_attn(0, 2)
        st_attn(0, 3)
        st_t1(1)
        st_uv(1)
        st_attn(1, 0)
        st_attn(1, 1)
        st_av(0)
        st_attn(1, 2)
        st_ffn_h(0)
        st_attn(1, 3)
        st_av(1)
        st_ffn_h(1)
        st_ffn_stats(0)
        st_ffn_stats(1)
        st_ffn_zo(0)
        st_ffn_zo(1)

    nc.compile()
    # Identity/Copy/Relu/Exp/Ln all live in table 6 (natural_log_exp);
    # the final two Sigmoids need table 2. The insertion pass picks
    # first-match sets per function and thrashes; retarget each load to the
    # table that covers everything up to the next load, then drop runs.
    SIG = mybir.ActivationFunctionType.Sigmoid
    for blk in nc.m.functions[0].blocks:
        insts = blk.instructions
        loads = [i for i, ins in enumerate(insts)
                 if isinstance(ins, mybir.InstLoadActFuncSet)]
        if not loads:
            continue
        assert all(insts[i].sync_info is None for i in loads)
        for k, li in enumerate(loads):
            end = loads[k + 1] if k + 1 < len(loads) else len(insts)
            funcs = {ins.func for ins in insts[li:end]
                     if isinstance(ins, mybir.InstActivation)}
            insts[li].act_func_set_id = 2 if SIG in funcs else 6
        keep_prev = None
        for li in reversed(loads):
            if keep_prev is not None and \
                    insts[li].act_func_set_id == keep_prev.act_func_set_id:
                blk.instructions.remove(keep_prev)
            keep_prev = insts[li]
    return nc


def _f16(a):
    return np.asarray(a, np.float32).astype(np.float16)


def _pack_weights(inputs):
    wA = np.zeros((128, WFA), np.float16)
    wB = np.zeros((128, WFB), np.float16)

    def put(name, arr):
        if name in WPACK_A:
            w, col, rows, cols = wA, *WPACK_A[name]
        else:
            w, col, rows, cols = wB, *WPACK_B[name]
        a = np.asarray(arr, np.float32).reshape(rows, cols)
        w[0:rows, col: col + cols] = a.astype(np.float16)

    f = {k: np.asarray(v, np.float32) for k, v in inputs.items()}
    put("Wp0", f["Wp"][:128])
    put("Wp1", f["Wp"][128:])
    Wqa = np.concatenate([f["Wq"], f["bq"].reshape(1, D)], 0)
    Wka = np.concatenate([f["Wk"], f["bk"].reshape(1, D)], 0)
    M1 = (Wqa @ Wka.T) / 8.0
    put("M1T", M1)  # lhsT: matmul computes lhsT.T @ rhs = M1.T @ xp~ = P^T scores
    put("Wv_a", np.concatenate([f["Wv"], f["bv"].reshape(1, D)], 0))
    put("Wf1", f["Wf1"])
    put("Wf2", f["Wf2"])
    put("Ws1p", f["ln_g"].reshape(D, 1) * f["Ws1"])
    put("Ws2", f["Ws2"])

    # f table and Toeplitz tiles (weight-only folding)
    dd = np.arange(MAXD + 1)
    emb = f["Eemb"][dd]
    wv = np.maximum(emb @ f["Wr1"] + f["br1"], 0.0) @ f["Wr2"]
    ftab = 1.0 / (1.0 + np.exp(-wv[:, 0]))
    f50 = ftab[MAXD]
    karr = np.arange(-1024, 2048)
    gp = np.where(np.abs(karr) <= MAXD,
                  ftab[np.clip(np.abs(karr), 0, MAXD)] - f50, 0.0)

    def gfun(k):
        return gp[k + 1024]

    a_i = np.arange(128).reshape(128, 1)
    b_i = np.arange(128).reshape(1, 128)
    for s, nm in ((-1, "Tm1"), (0, "T0"), (1, "Tp1")):
        put(nm, gfun(128 * s + a_i - b_i))
    put("ident", np.eye(128, dtype=np.float32))

    def put32(name, arr):
        if name in WPACK32_A:
            w, col, rows, cols = wA, *WPACK32_A[name]
        else:
            w, col, rows, cols = wB, *WPACK32_B[name]
        a = np.asarray(arr, np.float32).reshape(rows, cols)
        w[0:rows, col: col + 2 * cols] = a.view(np.float16)

    put32("bp", f["bp"])
    put32("bf1", f["bf1"])
    bs1p = f["ln_b"] @ f["Ws1"] + f["bs1"]
    put32("bs1p", bs1p.reshape(32, 1))
    put32("nbs2", np.broadcast_to(-f["bs2"].reshape(1, 1), (128, 1)))
    put32("f50", np.broadcast_to(np.float32(f50), (128, 1)))
    put32("m8", np.broadcast_to(np.float32(-8.0), (128, 1)))
    # sigma rides through the FFN (LN scale invariance), so var is scaled by
    # sigma^2 and the reference eps=1e-5 would dominate for small-sigma
    # tokens; eps only guards log(0), so make it negligible instead.
    put32("eps", np.broadcast_to(np.float32(1e-30), (128, 1)))
    return wA, wB


_NC_CACHE = {}


def _get_nc():
    if "nc" not in _NC_CACHE:
        _NC_CACHE["nc"] = build_nc()
    return _NC_CACHE["nc"]


def kernel(**inputs):
    from concourse.bass_utils import run_bass_kernel_spmd

    nc = _get_nc()

    x = np.asarray(inputs["x"], dtype=np.float32)
    r = np.asarray(inputs["price_rising_ranks"]).astype(np.int64)
    assert x.shape == (B, N, D_IN)

    wpkA, wpkB = _pack_weights(inputs)
    in_maps = []
    for core in range(N_CORES):
        rowpk = np.zeros((1, RF), np.float16)
        rowpk[0, ROW_ONES: ROW_ONES + N] = 1.0
        rowpk[0, ROW_BF2: ROW_BF2 + D] = _f16(inputs["bf2"]).reshape(D)
        rowpk[0, ROW_BF1: ROW_BF1 + 128] = _f16(inputs["bf1"]).reshape(128)
        xts = np.zeros((PB, 2, 128, N), np.float16)
        for b in range(PB):
            gb = core * PB + b
            xT = x[gb].T.astype(np.float16)  # [256, 512]
            xts[b, 0] = xT[:128]
            xts[b, 1] = xT[128:]
            rowpk[0, (ROW_R0 if b == 0 else ROW_R1):][: N] = \
                r[gb].astype(np.float16)
        in_maps.append({
            "xT": xts,
            "wpackA": wpkA,
            "wpackB": wpkB,
            "rowpack": rowpk,
        })

    res = run_bass_kernel_spmd(nc, in_maps, core_ids=list(range(N_CORES)))
    out = np.concatenate([res.results[c]["out"] for c in range(N_CORES)],
                         axis=0)
    return out.astype(np.float32)

